# revision 25
# baseline (speedup 1.0000x reference)
"""Trainium2 Bass kernel for a dense transformer block with sigmoid attention.

Shapes (hardcoded): B=8, N=1024, C=768, H=12 heads, D=64, HID=3072.
Sharding: data-parallel over batch -- one batch element per NeuronCore (8 cores).

Design (v3):
  - QK matmuls run 2 heads concurrently via PE row-tiling (64x128 tiles: head
    A's k/q on partitions 0:64, head B's on 64:128) -- no zero-padding.
  - AV matmuls run 2 heads concurrently via PE col-tiling (128x64 tiles: head
    A drains to psum partitions 0:64, head B to 64:128) -- no junk rows.
  - sigmoid(s) is computed two ways, split by (query-half, key-chunk-pair) so
    the 12.6M-element stream lands on two engines at once:
      * ACT: sigmoid(s) = 0.5 + 0.5*tanh(s/2 + ab/2) (exact); tanh lands in
        fp16 (bf16 would lose sigmoid's bits near t=-1). The +0.5 folds into
        AV: o = 0.5*(sum t*v) + 0.5*colsum(v) over the tanh key-chunks.
      * DVE: sigmoid(s) ~= exp(s+ab) (scores sit below -5 where the relative
        gap is <1%) via a Schraudolph bit-trick: int16(s*K1+K2) bitcast to
        bf16 is 2*exp(s+ab) to ~3%; the 2x cancels the 0.5 evac scale.
    Everything ACT runs (tanh, silu, identity) lives in ONE activation table
    set (silu_and_others) -> no 2.7us table reloads mid-kernel.
  - gelu(z) ~= silu(1.702*z)/1.702 on ACT (scale is free, 1/1.702 folds into
    w2 host-side).
  - LN rstd = 1/sqrt(var+eps) via bit-trick + 1 Newton step on GPSIMD (keeps
    both the sqrt table set and the 8-op chain off ACT/DVE).
  - LN transposes (token-major -> feature-major) run on the DMA engines via
    dma_start_transpose (bf16); ACT/DVE convert bf16->fp8 for DoubleRow.
    PE does zero transposes.
  - x is DMA'd once into a persistent SBUF copy (used by LN1 and the proj
    residual) -- keeps the sync DMA queue free of mid-kernel stalls.
  - Phase order: A (LN1+v+colsum) -> per-head-pair q/k matmuls + attention on
    query half 0 -> attention half 1 interleaved with the tail (proj/LN2/MLP)
    on half 0 -> tail half 1. The interleave keeps PE fed while ACT/DVE chew
    the sigmoid stream.
  - matmuls in fp8 DoubleRow where the contraction allows (qkv/proj/mlp); the
    residual stream stays fp32. Both branches are scaled by layerscale ~1e-6,
    so branch-side low precision is invisible at the output.
"""

import os

import numpy as np
import ml_dtypes

B, N, C, H = 8, 1024, 768, 12
D = C // H           # 64
HID = 4 * C          # 3072
LN_EPS = 1e-5
P = 128
KC = C // P          # 6   C chunks
NT = N // P          # 8   token chunks
MHID = HID // P      # 24  hidden chunks
NCORES = 8
HPAIRS = H // 2      # 6 head pairs == feature chunks of q/k
GELU_A = 1.7015043497085571  # gelu(x) ~= silu(A*x)/A

# sigmoid engine split is by head parity: head A of each pair (psum/oT
# partitions 0:64) uses ACT tanh; head B (64:128) uses the DVE exp trick.
ALL_PAIRS = (0, 1, 2, 3)

BF16 = ml_dtypes.bfloat16

LAST_EXEC_TIME_NS = None
LAST_TRACE_PATH = None
LAST_RESULTS = None


def _build_program(attn_bias: float, has_vbias: bool, has_bproj: bool, has_b2: bool):
    import concourse.bass as bass
    import concourse.mybir as mybir
    import concourse.tile as tile
    from concourse import bacc
    from concourse.masks import make_identity
    from contextlib import ExitStack

    dt = mybir.dt
    FP32 = dt.float32
    BF = dt.bfloat16
    FP16 = dt.float16
    F8 = dt.float8e4
    I16 = dt.int16
    I32 = dt.int32
    I8 = dt.int8
    DR = mybir.MatmulPerfMode.DoubleRow
    AF = mybir.ActivationFunctionType
    OP = mybir.AluOpType

    LOG2E = 1.4426950408889634
    # Schraudolph exp -> fp8e4 bits: int8(s*K1 + K2) = fp8(512*exp(s+ab))
    EXP_K1 = LOG2E * 8.0
    EXP_K2 = (9.0 + 7.0 + attn_bias * LOG2E) * 8.0 - 0.75

    nc = bacc.Bacc("TRN2", debug=False, enable_asserts=False,
                   target_bir_lowering=False, num_devices=NCORES)

    x_d = nc.dram_tensor("x", [N, C], FP32, kind="ExternalInput").ap()
    wqkv_d = nc.dram_tensor("wqkv_t", [C, 3 * C], F8, kind="ExternalInput").ap()
    bqkv_d = nc.dram_tensor("bqkv", [P, 2 * KC], FP32, kind="ExternalInput").ap()
    vb_d = nc.dram_tensor("vbias", [C], FP32, kind="ExternalInput").ap()
    wproj_d = nc.dram_tensor("wproj_t", [C, C], F8, kind="ExternalInput").ap()
    bproj_d = nc.dram_tensor("bproj", [C], FP32, kind="ExternalInput").ap()
    w1_d = nc.dram_tensor("w1_t", [C, HID], F8, kind="ExternalInput").ap()
    b1_d = nc.dram_tensor("b1s", [P, MHID], FP32, kind="ExternalInput").ap()
    w2_d = nc.dram_tensor("w2_t", [HID, C], F8, kind="ExternalInput").ap()
    b2_d = nc.dram_tensor("b2", [C], FP32, kind="ExternalInput").ap()
    out_d = nc.dram_tensor("out", [N, C], FP32, kind="ExternalOutput").ap()

    def bcast_row(src_1d_ap, p=P):
        return bass.AP(tensor=src_1d_ap.tensor, offset=src_1d_ap.offset,
                       ap=[[0, p]] + list(src_1d_ap.ap))

    with ExitStack() as ctx:
        tc = ctx.enter_context(tile.TileContext(nc))

        consts = ctx.enter_context(tc.tile_pool(name="consts", bufs=1))
        stream = ctx.enter_context(tc.tile_pool(name="stream", bufs=3))
        stats_p = ctx.enter_context(tc.tile_pool(name="stats", bufs=4))
        arena = ctx.enter_context(tc.tile_pool(name="arena", bufs=1))
        st_pool = ctx.enter_context(tc.tile_pool(name="sT", bufs=2))

        # ---- x DMA'd once, first, into a persistent copy ----
        xfull = arena.tile([P, NT, C], FP32, tag="xfull", name="xfull")
        X_ENG = {0: "sync", 1: "scalar", 2: "gpsimd", 3: "sync", 4: "scalar",
                 5: "gpsimd", 6: "sync", 7: "scalar"}

        # ---- constants ----
        abh_sb = consts.tile([P, 1], FP32, tag="abh")
        nc.vector.memset(abh_sb, attn_bias * 0.5)
        bqkv_sb = consts.tile([P, 2 * KC], FP32, tag="bqkv")
        nc.sync.dma_start(out=bqkv_sb, in_=bqkv_d)
        b1s_sb = consts.tile([P, MHID], FP32, tag="b1s")
        nc.sync.dma_start(out=b1s_sb, in_=b1_d)
        ones_sb = consts.tile([P, 1], BF, tag="ones")
        nc.vector.memset(ones_sb, 1.0)
        ident = consts.tile([P, P], BF, tag="ident")
        make_identity(nc, ident)
        if has_vbias:
            vb_bc = consts.tile([P, C], FP32, tag="vb_bc")
            nc.gpsimd.dma_start(out=vb_bc, in_=bcast_row(vb_d))
        if has_bproj:
            bproj_bc = consts.tile([P, C], FP32, tag="bproj_bc")
            nc.gpsimd.dma_start(out=bproj_bc, in_=bcast_row(bproj_d))
        if has_b2:
            b2_bc = consts.tile([P, C], FP32, tag="b2_bc")
            nc.gpsimd.dma_start(out=b2_bc, in_=bcast_row(b2_d))

        # ---- weights (v-columns first: phase A needs only those) ----
        wqkv_sb = arena.tile([P, KC, 3 * C], F8, tag="t14", name="wqkv_sb")
        for k in range(KC):
            nc.gpsimd.dma_start(out=wqkv_sb[:, k, 2 * C:],
                                in_=wqkv_d[k * P:(k + 1) * P, 2 * C:])
        # x tiles: spread across the three DMA queues
        for i in range(NT):
            eng = getattr(nc, X_ENG[i])
            eng.dma_start(out=xfull[:, i, :], in_=x_d[i * P:(i + 1) * P, :])
        for k in range(KC):
            nc.gpsimd.dma_start(out=wqkv_sb[:, k, :2 * C],
                                in_=wqkv_d[k * P:(k + 1) * P, :2 * C])
        wproj_sb = arena.tile([P, KC, C], F8, tag="t4", name="wproj_sb")
        for k in range(KC):
            nc.gpsimd.dma_start(out=wproj_sb[:, k, :],
                                in_=wproj_d[k * P:(k + 1) * P, :])

        # ---- persistent activations ----
        hT = arena.tile([P, KC, N], F8, tag="t24", name="hT")
        qT = arena.tile([P, KC, N], BF, tag="qT", name="qT")
        kTp = arena.tile([P, H, N], BF, tag="kTp", name="kTp")
        v_sb = arena.tile([P, NT, C + D], F8, tag="v", name="v")
        oT = arena.tile([P, KC, N], F8, tag="oT", name="oT")
        x2 = arena.tile([P, NT, C], FP32, tag="x2", name="x2")
        h2T = arena.tile([P, KC, N], F8, tag="h2T", name="h2T")
        m1T = arena.tile([P, MHID, N], F8, tag="t24", name="m1T")
        # csum[0:64, j] = 0.5 * sum_m v[m, 128j : 128j+64] (tanh heads only)
        csum_sb = arena.tile([P, KC], FP32, tag="csum", name="csum")

        # ---- helpers ----
        def ln_stats(src_ap, mva, j):
            """bn stats of one token tile -> mva[:, j, :] = (mean, var)."""
            stats = stats_p.tile([P, 3, 6], FP32, tag="ln_stats")
            xg = src_ap.rearrange("p (g d) -> p g d", g=3)
            for g in range(3):
                nc.vector.bn_stats(out=stats[:, g, :], in_=xg[:, g, :])
            nc.vector.bn_aggr(out=mva[:, j, :], in_=stats)

        def rsqrt_grp(mva, n):
            """rstd[:, j] = 1/sqrt(var[:, j]+eps) for an n-tile group, on DVE
            (magic bit-trick + 1 Newton step; one 7-op chain per group)."""
            v1 = stats_p.tile([P, n], FP32, tag="ln_v1", name="v1")
            nc.vector.tensor_scalar(out=v1, in0=mva[:, :, 1:2], scalar1=LN_EPS,
                                    scalar2=None, op0=OP.add)
            hi = stats_p.tile([P, n], I32, tag="ln_hi", name="hi")
            nc.vector.tensor_scalar(out=hi, in0=v1.bitcast(I32), scalar1=1,
                                    scalar2=None, op0=OP.logical_shift_right)
            r0 = stats_p.tile([P, n], I32, tag="ln_r0", name="r0")
            nc.vector.tensor_scalar(out=r0, in0=hi, scalar1=-1,
                                    scalar2=0x5f3759df, op0=OP.mult, op1=OP.add)
            r0f = r0.bitcast(FP32)
            bb = stats_p.tile([P, n], FP32, tag="ln_bb", name="bb")
            nc.vector.tensor_tensor(out=bb, in0=r0f, in1=r0f, op=OP.mult)
            nc.vector.tensor_tensor(out=bb, in0=bb, in1=v1, op=OP.mult)
            nc.vector.tensor_scalar(out=bb, in0=bb, scalar1=-0.5, scalar2=1.5,
                                    op0=OP.mult, op1=OP.add)
            rs = stats_p.tile([P, n], FP32, tag="ln_rs", name="rs")
            nc.vector.tensor_tensor(out=rs, in0=r0f, in1=bb, op=OP.mult)
            return rs

        def ln_finish(i, j, src_ap, dstT, mva, rs, nm, cast_eng, ps_tr=None):
            """normalize + transpose (PE if ps_tr given, else DMA) + fp8 cast."""
            ht = stream.tile([P, C], BF, tag="ht", name=f"ht_{nm}{i}")
            nc.vector.tensor_scalar(out=ht, in0=src_ap, scalar1=mva[:, j, 0:1],
                                    scalar2=rs[:, j:j + 1],
                                    op0=OP.subtract, op1=OP.mult)
            if ps_tr is not None:
                for c in range(KC):
                    tp = ps_tr.tile([P, P], BF, tag="tr", name="tr_ps")
                    nc.tensor.transpose(tp, ht[:, c * P:(c + 1) * P], ident)
                    nc.scalar.activation(out=dstT[:, c, i * P:(i + 1) * P],
                                         in_=tp, func=AF.Identity)
                return
            htb = stream.tile([P, KC, P], BF, tag="htb", name=f"htb_{nm}{i}")
            nc.sync.dma_start_transpose(out=htb, in_=ht)
            if cast_eng == "act":
                nc.scalar.activation(out=dstT[:, :, i * P:(i + 1) * P],
                                     in_=htb, func=AF.Identity)
            else:
                nc.vector.tensor_copy(out=dstT[:, :, i * P:(i + 1) * P],
                                      in_=htb)

        def emit_v_mms(i, psA):
            """v = h @ Wv.T for token tile i (token-major out, fp8 DR)."""
            for half, nw in ((0, 512), (1, 256)):
                ps = psA.tile([P, 512], FP32, tag="v_ps", name="ps_v")
                for k in range(0, KC, 2):
                    nc.tensor.matmul(ps[:, :nw],
                                     lhsT=hT[:, k:k + 2, i * P:(i + 1) * P],
                                     rhs=wqkv_sb[:, k:k + 2,
                                                 2 * C + half * 512:
                                                 2 * C + half * 512 + nw],
                                     start=(k == 0), stop=(k == KC - 2),
                                     perf_mode=DR)
                dst = v_sb[:, i, half * 512:half * 512 + nw]
                if has_vbias:
                    nc.vector.tensor_tensor(
                        out=dst, in0=ps[:, :nw],
                        in1=vb_bc[:, half * 512:half * 512 + nw], op=OP.add)
                else:
                    nc.scalar.activation(out=dst, in_=ps[:, :nw],
                                         func=AF.Identity)

        def emit_csums(psA):
            """csum[:, j] = 0.5*sum_m v[m, :] over all keys (tanh-head fold)."""
            cs_ps = psA.tile([P, KC], FP32, tag="cs_ps", name="ps_cs")
            for j in range(KC):
                for mc in range(NT):
                    nc.tensor.matmul(cs_ps[:, j:j + 1],
                                     lhsT=v_sb[:, mc, j * P:(j + 1) * P],
                                     rhs=ones_sb,
                                     start=(mc == 0), stop=(mc == NT - 1))
            nc.vector.tensor_scalar(out=csum_sb[0:64, :], in0=cs_ps[0:64, :],
                                    scalar1=0.5, scalar2=None, op0=OP.mult)

        # ================= Phase A: LN1 + v + colsum =========================
        with tc.tile_pool(name="psA", bufs=2, space="PSUM") as psA:
            # ~24 back-to-back matmuls on a dummy tile flip the PE's HAM
            # clock gate to 8/8 before the real matmul stream arrives.
            warm_sb = consts.tile([P, 512], BF, tag="warm")
            nc.vector.memset(warm_sb, 0.0)
            nc.vector.memset(v_sb[:, :, C:], 0.0)
            nc.gpsimd.memset(kTp, 0.0)
            wps = psA.tile([P, 256], FP32, tag="warm_ps", name="wps")
            for w in range(130):
                nc.tensor.matmul(wps, lhsT=warm_sb[:, 0:P],
                                 rhs=warm_sb[:, 0:256],
                                 start=(w == 0), stop=(w == 129))
            nc.vector.tensor_copy(out=warm_sb[:, 0:256].bitcast(FP32), in_=wps[:, 0:128])
            for g in range(4):
                mva = stats_p.tile([P, 2, 2], FP32, tag="mva", name=f"mva{g}")
                for j in range(2):
                    ln_stats(xfull[:, 2 * g + j, :], mva, j)
                rs = rsqrt_grp(mva, 2)
                for j in range(2):
                    i = 2 * g + j
                    ln_finish(i, j, xfull[:, i, :], hT, mva, rs, "a", "act",
                              ps_tr=psA)
                    emit_v_mms(i, psA)
            emit_csums(psA)

        # ================= attention block ==================================
        def attn_block(hp, nh, psQK, psAV):
            """QK + sigmoid + AV for head pair hp on query half nh."""
            n0 = nh * 512
            pso_t = psAV.tile([P, 512], FP32, tag="avt", name=f"ps_t{hp}{nh}")
            pso_e = psAV.tile([P, 512], FP32, tag="ave", name=f"ps_e{hp}{nh}")
            for pr in ALL_PAIRS:
                sts = []
                for hx in range(2):
                    pq = psQK.tile([P, 2, 512], FP32, tag="qk", name=f"ps_s{hx}")
                    for j, mc in enumerate((2 * pr, 2 * pr + 1)):
                        nc.tensor.matmul(
                            pq[:, j, :],
                            lhsT=kTp[:, 2 * hp + hx, mc * P:(mc + 1) * P],
                            rhs=qT[:, hp, n0:n0 + 512],
                            start=True, stop=True)
                    sts.append(pq)
                # head A -> ACT tanh (fp16); head B -> DVE exp trick (fp8)
                st_t = st_pool.tile([P, 2, 512], FP16, tag="st_t", name="st0")
                nc.scalar.activation(out=st_t, in_=sts[0], func=AF.Tanh,
                                     bias=abh_sb, scale=0.5)
                st_e = st_pool.tile([P, 2, 512], F8, tag="st_e", name="st1")
                nc.vector.tensor_scalar(out=st_e.bitcast(I8), in0=sts[1],
                                        scalar1=EXP_K1, scalar2=EXP_K2,
                                        op0=OP.mult, op1=OP.add)
                # AV: tanh head 2 plain MMs (junk rows 64:), exp head 1 DR MM
                for j, mc in enumerate((2 * pr, 2 * pr + 1)):
                    nc.tensor.matmul(
                        pso_t,
                        lhsT=v_sb[:, mc, hp * P:hp * P + P],
                        rhs=st_t[:, j, :],
                        start=(mc == 0), stop=(mc == NT - 1))
                nc.tensor.matmul(
                    pso_e,
                    lhsT=v_sb[:, 2 * pr:2 * pr + 2, hp * P + 64:hp * P + 192],
                    rhs=st_e, perf_mode=DR,
                    start=(pr == 0), stop=(pr == len(ALL_PAIRS) - 1))
            if nh == 0:
                nc.vector.tensor_scalar(out=oT[0:64, hp, n0:n0 + 512],
                                        in0=pso_t[0:64, :], scalar1=0.5,
                                        scalar2=csum_sb[0:64, hp:hp + 1],
                                        op0=OP.mult, op1=OP.add)
                nc.vector.tensor_scalar(out=oT[64:P, hp, n0:n0 + 512],
                                        in0=pso_e[0:64, :], scalar1=1.0 / 512.0,
                                        scalar2=None, op0=OP.mult)
            else:
                nc.scalar.activation(out=oT[0:64, hp, n0:n0 + 512],
                                     in_=pso_t[0:64, :], func=AF.Identity,
                                     scale=0.5, bias=csum_sb[0:64, hp:hp + 1])
                nc.scalar.activation(out=oT[64:P, hp, n0:n0 + 512],
                                     in_=pso_e[0:64, :], func=AF.Identity,
                                     scale=1.0 / 512.0)

        # ================= tail unit builders ================================
        mvb = {}

        def proj_resid(i, psT):
            """x2 = x + proj(oT) for token tile i, plus its LN2 stats."""
            for half, nw in ((0, 512), (1, 256)):
                ps = psT.tile([P, 512], FP32, tag="m1a" if half == 0 else "m1b",
                          name="ps_c")
                for k in range(0, KC, 2):
                    nc.tensor.matmul(ps[:, :nw],
                                     lhsT=oT[:, k:k + 2, i * P:(i + 1) * P],
                                     rhs=wproj_sb[:, k:k + 2,
                                                  half * 512:half * 512 + nw],
                                     start=(k == 0), stop=(k == KC - 2),
                                     perf_mode=DR)
                dst = x2[:, i, half * 512:half * 512 + nw]
                nc.vector.tensor_tensor(
                    out=dst, in0=ps[:, :nw],
                    in1=xfull[:, i, half * 512:half * 512 + nw], op=OP.add)
                if has_bproj:
                    nc.vector.tensor_tensor(
                        out=dst, in0=dst,
                        in1=bproj_bc[:, half * 512:half * 512 + nw], op=OP.add)
            g = i // 4
            if i % 4 == 0:
                mvb[g] = stats_p.tile([P, 4, 2], FP32, tag="mvb", name=f"mvb{g}")
            ln_stats(x2[:, i, :], mvb[g], i % 4)

        def ln2_group(g, psT=None):
            """LN2 normalize+transpose+cast for token tiles 4g..4g+3.
            psT given -> transpose on PE via the m1a/m1b psum slots."""
            rs = rsqrt_grp(mvb[g], 4)
            for j in range(4):
                i = 4 * g + j
                if psT is None:
                    ln_finish(i, j, x2[:, i, :], h2T, mvb[g], rs, "b", "act")
                    continue
                ht = stream.tile([P, C], BF, tag="ht", name=f"ht_b{i}")
                nc.vector.tensor_scalar(out=ht, in0=x2[:, i, :],
                                        scalar1=mvb[g][:, j, 0:1],
                                        scalar2=rs[:, j:j + 1],
                                        op0=OP.subtract, op1=OP.mult)
                for c in range(KC):
                    tpb = psT.tile([P, 1024], BF,
                                   tag="m1a" if c % 2 == 0 else "m1b",
                                   name="tr2_ps")
                    nc.tensor.transpose(tpb[:, 0:P], ht[:, c * P:(c + 1) * P],
                                        ident)
                    nc.scalar.activation(out=h2T[:, c, i * P:(i + 1) * P],
                                         in_=tpb[:, 0:P], func=AF.Identity)

        def mlp1_chunk(mc, nh, psT):
            nsl = slice(nh * 512, (nh + 1) * 512)
            ps = psT.tile([P, 512], FP32, tag="m1a" if mc % 2 == 0 else "m1b",
                          name="ps_m1")
            for k in range(0, KC, 2):
                nc.tensor.matmul(ps,
                                 lhsT=w1_sb[:, k:k + 2, mc * P:(mc + 1) * P],
                                 rhs=h2T[:, k:k + 2, nsl],
                                 start=(k == 0), stop=(k == KC - 2),
                                 perf_mode=DR)
            nc.scalar.activation(out=m1T[:, mc, nsl], in_=ps, func=AF.Silu,
                                 bias=b1s_sb[:, mc:mc + 1], scale=GELU_A)

        def mlp2_tile(i, psT):
            ot = stream.tile([P, C], FP32, tag="io_o", name=f"out_t{i}")
            for half, nw in ((0, 512), (1, 256)):
                ps = psT.tile([P, 512], FP32, tag="m1a" if half == 0 else "m1b",
                          name="ps_m2")
                for k in range(0, MHID, 2):
                    nc.tensor.matmul(ps[:, :nw],
                                     lhsT=m1T[:, k:k + 2, i * P:(i + 1) * P],
                                     rhs=w2_sb[:, k:k + 2,
                                               half * 512:half * 512 + nw],
                                     start=(k == 0), stop=(k == MHID - 2),
                                     perf_mode=DR)
                dst = ot[:, half * 512:half * 512 + nw]
                nc.vector.tensor_tensor(out=dst, in0=ps[:, :nw],
                                        in1=x2[:, i, half * 512:half * 512 + nw],
                                        op=OP.add)
                if has_b2:
                    nc.vector.tensor_tensor(
                        out=dst, in0=dst,
                        in1=b2_bc[:, half * 512:half * 512 + nw], op=OP.add)
            nc.gpsimd.dma_start(out=out_d[i * P:(i + 1) * P, :], in_=ot)

        # ============ A2 (q/k chunk matmuls) + B1 / B2||T1 / T2 =============
        w1_sb = arena.tile([P, KC, HID], F8, tag="t18", name="w1_sb")
        w2_sb = arena.tile([P, MHID, C], F8, tag="t18b", name="w2_sb")

        with tc.tile_pool(name="psQK", bufs=2, space="PSUM") as psQK, \
             tc.tile_pool(name="psAV", bufs=1, space="PSUM") as psAV:
            with tc.tile_pool(name="psA2", bufs=1, space="PSUM") as psA2:
                for hp in range(HPAIRS):
                    # q then k chunk matmul; evac on ACT (identity + bias)
                    for which, dstT, col0 in ((0, qT, hp * P),
                                              (1, kTp, C + hp * P)):
                        ps2 = psA2.tile([P, N], FP32, tag="qk2", name="ps_qk2")
                        for half in range(2):
                            for k in range(0, KC, 2):
                                nc.tensor.matmul(
                                    ps2[:, half * 512:(half + 1) * 512],
                                    lhsT=wqkv_sb[:, k:k + 2, col0:col0 + P],
                                    rhs=hT[:, k:k + 2,
                                           half * 512:(half + 1) * 512],
                                    start=(k == 0), stop=(k == KC - 2),
                                    perf_mode=DR)
                        bcol = hp if which == 0 else KC + hp
                        if which == 0:
                            nc.scalar.activation(out=dstT[:, hp, :], in_=ps2,
                                                 func=AF.Identity,
                                                 bias=bqkv_sb[:, bcol:bcol + 1])
                        else:
                            nc.scalar.activation(
                                out=dstT[0:D, 2 * hp, :], in_=ps2[0:D, :],
                                func=AF.Identity,
                                bias=bqkv_sb[0:D, bcol:bcol + 1])
                            nc.scalar.activation(
                                out=dstT[D:P, 2 * hp + 1, :], in_=ps2[D:P, :],
                                func=AF.Identity,
                                bias=bqkv_sb[D:P, bcol:bcol + 1])
                    attn_block(hp, 0, psQK, psAV)
                    if hp == 0:
                        for k in range(KC):
                            nc.gpsimd.dma_start(out=w1_sb[:, k, :],
                                                in_=w1_d[k * P:(k + 1) * P, :])
                    if hp == 2:
                        for k in range(MHID):
                            nc.gpsimd.dma_start(out=w2_sb[:, k, :],
                                                in_=w2_d[k * P:(k + 1) * P, :])

            with tc.tile_pool(name="psT", bufs=1, space="PSUM") as psT:
                # B2 interleaved with T1 (tail on query half 0)
                tail_units = (
                    [lambda i=i: proj_resid(i, psT) for i in range(4)]
                    + [lambda: ln2_group(0)]
                    + [lambda mc=mc: mlp1_chunk(mc, 0, psT) for mc in range(MHID)]
                    + [lambda i=i: mlp2_tile(i, psT) for i in range(4)]
                )
                bounds = [0, 2, 5, 13, 21, 30, 31]
                for hp in range(HPAIRS):
                    attn_block(hp, 1, psQK, psAV)
                    for u in range(bounds[hp], bounds[hp + 1]):
                        tail_units[u]()
                # T2: tail on query half 1 (T1's last mlp2 tiles bridge the
                # proj/LN2 dependency chain)
                proj_resid(4, psT)
                proj_resid(5, psT)
                mlp2_tile(2, psT)
                proj_resid(6, psT)
                proj_resid(7, psT)
                mlp2_tile(3, psT)
                ln2_group(1, psT)
                for mc in range(MHID):
                    mlp1_chunk(mc, 1, psT)
                for i in range(4, NT):
                    mlp2_tile(i, psT)

    nc.finalize()
    return nc


def kernel(x, ln1_w, ln1_b, qkv_w, qkv_b, proj_w, proj_b, attn_bias,
           ls1, ln2_w, ln2_b, w1, b1, w2, b2, ls2):
    global LAST_EXEC_TIME_NS, LAST_TRACE_PATH, LAST_RESULTS
    from concourse.bass_utils import run_bass_kernel_spmd

    x = np.asarray(x, np.float32)
    f32 = lambda a: np.asarray(a, np.float32)
    ln1_w, ln1_b, qkv_w, qkv_b = f32(ln1_w), f32(ln1_b), f32(qkv_w), f32(qkv_b)
    proj_w, proj_b, ls1 = f32(proj_w), f32(proj_b), f32(ls1)
    ln2_w, ln2_b, w1, b1, w2, b2, ls2 = (f32(ln2_w), f32(ln2_b), f32(w1),
                                         f32(b1), f32(w2), f32(b2), f32(ls2))
    ab = float(np.asarray(attn_bias, np.float32))

    # ---- host-side weight folding (fp32, then cast to fp8) ----
    scale = D ** -0.5
    qkv_w_eff = qkv_w * ln1_w[None, :]
    bqkv_eff = qkv_b + qkv_w @ ln1_b
    wqkv_t = np.ascontiguousarray(qkv_w_eff.T)
    wqkv_t[:, :C] *= scale
    bqkv_eff = bqkv_eff.copy()
    bqkv_eff[:C] *= scale
    wproj_t = np.ascontiguousarray((proj_w * ls1[:, None]).T)
    bproj_eff = proj_b * ls1
    w1_t = np.ascontiguousarray((w1 * ln2_w[None, :]).T)
    b1_eff = (b1 + w1 @ ln2_b) * GELU_A          # silu input bias, pre-scaled
    w2_t = np.ascontiguousarray((w2 * ls2[:, None]).T) / GELU_A
    b2_eff = b2 * ls2

    has_vbias = bool(np.any(bqkv_eff[2 * C:] != 0.0))
    has_bproj = bool(np.any(bproj_eff != 0.0))
    has_b2 = bool(np.any(b2_eff != 0.0))

    nc = _build_program(ab, has_vbias, has_bproj, has_b2)

    import concourse.mybir as mybir
    F8NP = mybir.dt.np(mybir.dt.float8e4)
    shared = {
        "wqkv_t": wqkv_t.astype(F8NP),
        "bqkv": np.ascontiguousarray(
            bqkv_eff[:2 * C].reshape(2 * KC, P).T).astype(np.float32),
        "vbias": bqkv_eff[2 * C:].astype(np.float32),
        "wproj_t": wproj_t.astype(F8NP),
        "bproj": bproj_eff.astype(np.float32),
        "w1_t": w1_t.astype(F8NP),
        "b1s": np.ascontiguousarray(
            b1_eff.reshape(MHID, P).T).astype(np.float32),
        "w2_t": w2_t.astype(F8NP),
        "b2": b2_eff.astype(np.float32),
    }
    in_maps = [dict(shared, x=np.ascontiguousarray(x[c])) for c in range(NCORES)]

    trace = os.environ.get("KERNEL_TRACE", "0") == "1"
    res = run_bass_kernel_spmd(nc, in_maps, core_ids=list(range(NCORES)),
                               trace=trace)
    LAST_EXEC_TIME_NS = res.exec_time_ns
    LAST_RESULTS = res
    if res.instructions_and_trace is not None:
        LAST_TRACE_PATH = res.instructions_and_trace[1]
    return np.stack([r["out"] for r in res.results]).astype(np.float32)


# revision 26
# speedup vs baseline: 1.0308x; 1.0308x over previous
"""Trainium2 Bass kernel for a dense transformer block with sigmoid attention.

Shapes (hardcoded): B=8, N=1024, C=768, H=12 heads, D=64, HID=3072.
Sharding: data-parallel over batch -- one batch element per NeuronCore (8 cores).

Design (v3):
  - QK matmuls run 2 heads concurrently via PE row-tiling (64x128 tiles: head
    A's k/q on partitions 0:64, head B's on 64:128) -- no zero-padding.
  - AV matmuls run 2 heads concurrently via PE col-tiling (128x64 tiles: head
    A drains to psum partitions 0:64, head B to 64:128) -- no junk rows.
  - sigmoid(s) is computed two ways, split by (query-half, key-chunk-pair) so
    the 12.6M-element stream lands on two engines at once:
      * ACT: sigmoid(s) = 0.5 + 0.5*tanh(s/2 + ab/2) (exact); tanh lands in
        fp16 (bf16 would lose sigmoid's bits near t=-1). The +0.5 folds into
        AV: o = 0.5*(sum t*v) + 0.5*colsum(v) over the tanh key-chunks.
      * DVE: sigmoid(s) ~= exp(s+ab) (scores sit below -5 where the relative
        gap is <1%) via a Schraudolph bit-trick: int16(s*K1+K2) bitcast to
        bf16 is 2*exp(s+ab) to ~3%; the 2x cancels the 0.5 evac scale.
    Everything ACT runs (tanh, silu, identity) lives in ONE activation table
    set (silu_and_others) -> no 2.7us table reloads mid-kernel.
  - gelu(z) ~= silu(1.702*z)/1.702 on ACT (scale is free, 1/1.702 folds into
    w2 host-side).
  - LN rstd = 1/sqrt(var+eps) via bit-trick + 1 Newton step on GPSIMD (keeps
    both the sqrt table set and the 8-op chain off ACT/DVE).
  - LN transposes (token-major -> feature-major) run on the DMA engines via
    dma_start_transpose (bf16); ACT/DVE convert bf16->fp8 for DoubleRow.
    PE does zero transposes.
  - x is DMA'd once into a persistent SBUF copy (used by LN1 and the proj
    residual) -- keeps the sync DMA queue free of mid-kernel stalls.
  - Phase order: A (LN1+v+colsum) -> per-head-pair q/k matmuls + attention on
    query half 0 -> attention half 1 interleaved with the tail (proj/LN2/MLP)
    on half 0 -> tail half 1. The interleave keeps PE fed while ACT/DVE chew
    the sigmoid stream.
  - matmuls in fp8 DoubleRow where the contraction allows (qkv/proj/mlp); the
    residual stream stays fp32. Both branches are scaled by layerscale ~1e-6,
    so branch-side low precision is invisible at the output.
"""

import os

import numpy as np
import ml_dtypes

B, N, C, H = 8, 1024, 768, 12
D = C // H           # 64
HID = 4 * C          # 3072
LN_EPS = 1e-5
P = 128
KC = C // P          # 6   C chunks
NT = N // P          # 8   token chunks
MHID = HID // P      # 24  hidden chunks
NCORES = 8
HPAIRS = H // 2      # 6 head pairs == feature chunks of q/k
GELU_A = 1.7015043497085571  # gelu(x) ~= silu(A*x)/A

# sigmoid engine split is by head parity: head A of each pair (psum/oT
# partitions 0:64) uses ACT tanh; head B (64:128) uses the DVE exp trick.
ALL_PAIRS = (0, 1, 2, 3)

BF16 = ml_dtypes.bfloat16

LAST_EXEC_TIME_NS = None
LAST_TRACE_PATH = None
LAST_RESULTS = None


def _build_program(attn_bias: float, has_vbias: bool, has_bproj: bool, has_b2: bool):
    import concourse.bass as bass
    import concourse.mybir as mybir
    import concourse.tile as tile
    from concourse import bacc
    from concourse.masks import make_identity
    from contextlib import ExitStack

    dt = mybir.dt
    FP32 = dt.float32
    BF = dt.bfloat16
    FP16 = dt.float16
    F8 = dt.float8e4
    I16 = dt.int16
    I32 = dt.int32
    I8 = dt.int8
    DR = mybir.MatmulPerfMode.DoubleRow
    AF = mybir.ActivationFunctionType
    OP = mybir.AluOpType

    LOG2E = 1.4426950408889634
    # Schraudolph exp -> fp8e4 bits: int8(s*K1 + K2) = fp8(512*exp(s+ab))
    EXP_K1 = LOG2E * 8.0
    EXP_K2 = (9.0 + 7.0 + attn_bias * LOG2E) * 8.0 - 0.75

    nc = bacc.Bacc("TRN2", debug=False, enable_asserts=False,
                   target_bir_lowering=False, num_devices=NCORES)

    x_d = nc.dram_tensor("x", [N, C], FP32, kind="ExternalInput").ap()
    wqkv_d = nc.dram_tensor("wqkv_t", [C, 3 * C], F8, kind="ExternalInput").ap()
    bqkv_d = nc.dram_tensor("bqkv", [P, 2 * KC], FP32, kind="ExternalInput").ap()
    vb_d = nc.dram_tensor("vbias", [C], FP32, kind="ExternalInput").ap()
    wproj_d = nc.dram_tensor("wproj_t", [C, C], F8, kind="ExternalInput").ap()
    bproj_d = nc.dram_tensor("bproj", [C], FP32, kind="ExternalInput").ap()
    w1_d = nc.dram_tensor("w1_t", [C, HID], F8, kind="ExternalInput").ap()
    b1_d = nc.dram_tensor("b1s", [P, MHID], FP32, kind="ExternalInput").ap()
    w2_d = nc.dram_tensor("w2_t", [HID, C], F8, kind="ExternalInput").ap()
    b2_d = nc.dram_tensor("b2", [C], FP32, kind="ExternalInput").ap()
    out_d = nc.dram_tensor("out", [N, C], FP32, kind="ExternalOutput").ap()

    def bcast_row(src_1d_ap, p=P):
        return bass.AP(tensor=src_1d_ap.tensor, offset=src_1d_ap.offset,
                       ap=[[0, p]] + list(src_1d_ap.ap))

    with ExitStack() as ctx:
        tc = ctx.enter_context(tile.TileContext(nc))

        consts = ctx.enter_context(tc.tile_pool(name="consts", bufs=1))
        stream = ctx.enter_context(tc.tile_pool(name="stream", bufs=3))
        stats_p = ctx.enter_context(tc.tile_pool(name="stats", bufs=4))
        arena = ctx.enter_context(tc.tile_pool(name="arena", bufs=1))
        st_pool = ctx.enter_context(tc.tile_pool(name="sT", bufs=2))

        # ---- x DMA'd once, first, into a persistent copy ----
        xfull = arena.tile([P, NT, C], FP32, tag="xfull", name="xfull")

        # ---- constants ----
        abh_sb = consts.tile([P, 1], FP32, tag="abh")
        nc.vector.memset(abh_sb, attn_bias * 0.5)
        bqkv_sb = consts.tile([P, 2 * KC], FP32, tag="bqkv")
        nc.sync.dma_start(out=bqkv_sb, in_=bqkv_d)
        b1s_sb = consts.tile([P, MHID], FP32, tag="b1s")
        nc.sync.dma_start(out=b1s_sb, in_=b1_d)
        ones_sb = consts.tile([P, 1], BF, tag="ones")
        nc.vector.memset(ones_sb, 1.0)
        ident = consts.tile([P, P], BF, tag="ident")
        make_identity(nc, ident)
        if has_vbias:
            vb_bc = consts.tile([P, C], FP32, tag="vb_bc")
            nc.gpsimd.dma_start(out=vb_bc, in_=bcast_row(vb_d))
        if has_bproj:
            bproj_bc = consts.tile([P, C], FP32, tag="bproj_bc")
            nc.gpsimd.dma_start(out=bproj_bc, in_=bcast_row(bproj_d))
        if has_b2:
            b2_bc = consts.tile([P, C], FP32, tag="b2_bc")
            nc.gpsimd.dma_start(out=b2_bc, in_=bcast_row(b2_d))

        # ---- weights (v-columns first: phase A needs only those) ----
        wqkv_sb = arena.tile([P, KC, 3 * C], F8, tag="t14", name="wqkv_sb")
        for k in range(KC):
            nc.gpsimd.dma_start(out=wqkv_sb[:, k, 2 * C:],
                                in_=wqkv_d[k * P:(k + 1) * P, 2 * C:])
        # x tiles: each tile split by columns across the three DMA queues
        for i in range(NT):
            for q, (eng, c0, c1) in enumerate(
                    ((nc.sync, 0, 256), (nc.scalar, 256, 512),
                     (nc.gpsimd, 512, 768))):
                eng.dma_start(out=xfull[:, i, c0:c1],
                              in_=x_d[i * P:(i + 1) * P, c0:c1])
        for k in range(KC):
            nc.gpsimd.dma_start(out=wqkv_sb[:, k, :2 * C],
                                in_=wqkv_d[k * P:(k + 1) * P, :2 * C])
        wproj_sb = arena.tile([P, KC, C], F8, tag="t4", name="wproj_sb")
        for k in range(KC):
            nc.gpsimd.dma_start(out=wproj_sb[:, k, :],
                                in_=wproj_d[k * P:(k + 1) * P, :])

        # ---- persistent activations ----
        hT = arena.tile([P, KC, N], F8, tag="t24", name="hT")
        qT = arena.tile([P, KC, N], BF, tag="qT", name="qT")
        kTp = arena.tile([P, H, N], BF, tag="kTp", name="kTp")
        v_sb = arena.tile([P, NT, C + D], F8, tag="v", name="v")
        oT = arena.tile([P, KC, N], F8, tag="oT", name="oT")
        x2 = arena.tile([P, NT, C], FP32, tag="x2", name="x2")
        h2T = arena.tile([P, KC, N], F8, tag="h2T", name="h2T")
        m1T = arena.tile([P, MHID, N], F8, tag="t24", name="m1T")
        # csum[0:64, j] = 0.5 * sum_m v[m, 128j : 128j+64] (tanh heads only)
        csum_sb = arena.tile([P, KC], FP32, tag="csum", name="csum")

        # ---- helpers ----
        def ln_stats(src_ap, mva, j):
            """bn stats of one token tile -> mva[:, j, :] = (mean, var)."""
            stats = stats_p.tile([P, 3, 6], FP32, tag="ln_stats")
            xg = src_ap.rearrange("p (g d) -> p g d", g=3)
            for g in range(3):
                nc.vector.bn_stats(out=stats[:, g, :], in_=xg[:, g, :])
            nc.vector.bn_aggr(out=mva[:, j, :], in_=stats)

        def rsqrt_grp(mva, n):
            """rstd[:, j] = 1/sqrt(var[:, j]+eps) for an n-tile group, on DVE
            (magic bit-trick + 1 Newton step; one 7-op chain per group)."""
            v1 = stats_p.tile([P, n], FP32, tag="ln_v1", name="v1")
            nc.vector.tensor_scalar(out=v1, in0=mva[:, :, 1:2], scalar1=LN_EPS,
                                    scalar2=None, op0=OP.add)
            hi = stats_p.tile([P, n], I32, tag="ln_hi", name="hi")
            nc.vector.tensor_scalar(out=hi, in0=v1.bitcast(I32), scalar1=1,
                                    scalar2=None, op0=OP.logical_shift_right)
            r0 = stats_p.tile([P, n], I32, tag="ln_r0", name="r0")
            nc.vector.tensor_scalar(out=r0, in0=hi, scalar1=-1,
                                    scalar2=0x5f3759df, op0=OP.mult, op1=OP.add)
            r0f = r0.bitcast(FP32)
            bb = stats_p.tile([P, n], FP32, tag="ln_bb", name="bb")
            nc.vector.tensor_tensor(out=bb, in0=r0f, in1=r0f, op=OP.mult)
            nc.vector.tensor_tensor(out=bb, in0=bb, in1=v1, op=OP.mult)
            nc.vector.tensor_scalar(out=bb, in0=bb, scalar1=-0.5, scalar2=1.5,
                                    op0=OP.mult, op1=OP.add)
            rs = stats_p.tile([P, n], FP32, tag="ln_rs", name="rs")
            nc.vector.tensor_tensor(out=rs, in0=r0f, in1=bb, op=OP.mult)
            return rs

        def ln_finish(i, j, src_ap, dstT, mva, rs, nm, cast_eng, ps_tr=None):
            """normalize + transpose (PE if ps_tr given, else DMA) + fp8 cast."""
            ht = stream.tile([P, C], BF, tag="ht", name=f"ht_{nm}{i}")
            nc.vector.tensor_scalar(out=ht, in0=src_ap, scalar1=mva[:, j, 0:1],
                                    scalar2=rs[:, j:j + 1],
                                    op0=OP.subtract, op1=OP.mult)
            if ps_tr is not None:
                for c in range(KC):
                    tp = ps_tr.tile([P, P], BF, tag="tr", name="tr_ps")
                    nc.tensor.transpose(tp, ht[:, c * P:(c + 1) * P], ident)
                    nc.scalar.activation(out=dstT[:, c, i * P:(i + 1) * P],
                                         in_=tp, func=AF.Identity)
                return
            htb = stream.tile([P, KC, P], BF, tag="htb", name=f"htb_{nm}{i}")
            nc.sync.dma_start_transpose(out=htb, in_=ht)
            if cast_eng == "act":
                nc.scalar.activation(out=dstT[:, :, i * P:(i + 1) * P],
                                     in_=htb, func=AF.Identity)
            else:
                nc.vector.tensor_copy(out=dstT[:, :, i * P:(i + 1) * P],
                                      in_=htb)

        def emit_v_mms(i, psA):
            """v = h @ Wv.T for token tile i (token-major out, fp8 DR)."""
            for half, nw in ((0, 512), (1, 256)):
                ps = psA.tile([P, 512], FP32, tag="v_ps", name="ps_v")
                for k in range(0, KC, 2):
                    nc.tensor.matmul(ps[:, :nw],
                                     lhsT=hT[:, k:k + 2, i * P:(i + 1) * P],
                                     rhs=wqkv_sb[:, k:k + 2,
                                                 2 * C + half * 512:
                                                 2 * C + half * 512 + nw],
                                     start=(k == 0), stop=(k == KC - 2),
                                     perf_mode=DR)
                dst = v_sb[:, i, half * 512:half * 512 + nw]
                if has_vbias:
                    nc.vector.tensor_tensor(
                        out=dst, in0=ps[:, :nw],
                        in1=vb_bc[:, half * 512:half * 512 + nw], op=OP.add)
                else:
                    nc.scalar.activation(out=dst, in_=ps[:, :nw],
                                         func=AF.Identity)

        def emit_csums(psA):
            """csum[:, j] = 0.5*sum_m v[m, :] over all keys (tanh-head fold)."""
            cs_ps = psA.tile([P, KC], FP32, tag="cs_ps", name="ps_cs")
            for j in range(KC):
                for mc in range(NT):
                    nc.tensor.matmul(cs_ps[:, j:j + 1],
                                     lhsT=v_sb[:, mc, j * P:(j + 1) * P],
                                     rhs=ones_sb,
                                     start=(mc == 0), stop=(mc == NT - 1))
            nc.vector.tensor_scalar(out=csum_sb[0:64, :], in0=cs_ps[0:64, :],
                                    scalar1=0.5, scalar2=None, op0=OP.mult)

        # ================= Phase A: LN1 + v + colsum =========================
        with tc.tile_pool(name="psA", bufs=2, space="PSUM") as psA:
            # ~24 back-to-back matmuls on a dummy tile flip the PE's HAM
            # clock gate to 8/8 before the real matmul stream arrives.
            warm_sb = consts.tile([P, 512], BF, tag="warm")
            nc.vector.memset(warm_sb, 0.0)
            nc.vector.memset(v_sb[:, :, C:], 0.0)
            nc.gpsimd.memset(kTp, 0.0)
            wps = psA.tile([P, 256], FP32, tag="warm_ps", name="wps")
            for w in range(60):
                nc.tensor.matmul(wps, lhsT=warm_sb[:, 0:P],
                                 rhs=warm_sb[:, 0:256],
                                 start=(w == 0), stop=(w == 59))
            nc.vector.tensor_copy(out=warm_sb[:, 0:256].bitcast(FP32), in_=wps[:, 0:128])
            for g in range(4):
                mva = stats_p.tile([P, 2, 2], FP32, tag="mva", name=f"mva{g}")
                for j in range(2):
                    ln_stats(xfull[:, 2 * g + j, :], mva, j)
                rs = rsqrt_grp(mva, 2)
                for j in range(2):
                    i = 2 * g + j
                    ln_finish(i, j, xfull[:, i, :], hT, mva, rs, "a", "act",
                              ps_tr=psA)
                    emit_v_mms(i, psA)
            emit_csums(psA)

        # ================= attention block ==================================
        def attn_block(hp, nh, psQK, psAV):
            """QK + sigmoid + AV for head pair hp on query half nh."""
            n0 = nh * 512
            pso_t = psAV.tile([P, 512], FP32, tag="avt", name=f"ps_t{hp}{nh}")
            pso_e = psAV.tile([P, 512], FP32, tag="ave", name=f"ps_e{hp}{nh}")
            for pr in ALL_PAIRS:
                sts = []
                for hx in range(2):
                    pq = psQK.tile([P, 2, 512], FP32, tag="qk", name=f"ps_s{hx}")
                    for j, mc in enumerate((2 * pr, 2 * pr + 1)):
                        nc.tensor.matmul(
                            pq[:, j, :],
                            lhsT=kTp[:, 2 * hp + hx, mc * P:(mc + 1) * P],
                            rhs=qT[:, hp, n0:n0 + 512],
                            start=True, stop=True)
                    sts.append(pq)
                # head A -> ACT tanh (fp16); head B -> DVE exp trick (fp8)
                st_t = st_pool.tile([P, 2, 512], FP16, tag="st_t", name="st0")
                nc.scalar.activation(out=st_t, in_=sts[0], func=AF.Tanh,
                                     bias=abh_sb, scale=0.5)
                st_e = st_pool.tile([P, 2, 512], F8, tag="st_e", name="st1")
                nc.vector.tensor_scalar(out=st_e.bitcast(I8), in0=sts[1],
                                        scalar1=EXP_K1, scalar2=EXP_K2,
                                        op0=OP.mult, op1=OP.add)
                # AV: tanh head 2 plain MMs (junk rows 64:), exp head 1 DR MM
                for j, mc in enumerate((2 * pr, 2 * pr + 1)):
                    nc.tensor.matmul(
                        pso_t,
                        lhsT=v_sb[:, mc, hp * P:hp * P + P],
                        rhs=st_t[:, j, :],
                        start=(mc == 0), stop=(mc == NT - 1))
                nc.tensor.matmul(
                    pso_e,
                    lhsT=v_sb[:, 2 * pr:2 * pr + 2, hp * P + 64:hp * P + 192],
                    rhs=st_e, perf_mode=DR,
                    start=(pr == 0), stop=(pr == len(ALL_PAIRS) - 1))
            if nh == 0:
                nc.vector.tensor_scalar(out=oT[0:64, hp, n0:n0 + 512],
                                        in0=pso_t[0:64, :], scalar1=0.5,
                                        scalar2=csum_sb[0:64, hp:hp + 1],
                                        op0=OP.mult, op1=OP.add)
                nc.vector.tensor_scalar(out=oT[64:P, hp, n0:n0 + 512],
                                        in0=pso_e[0:64, :], scalar1=1.0 / 512.0,
                                        scalar2=None, op0=OP.mult)
            else:
                nc.scalar.activation(out=oT[0:64, hp, n0:n0 + 512],
                                     in_=pso_t[0:64, :], func=AF.Identity,
                                     scale=0.5, bias=csum_sb[0:64, hp:hp + 1])
                nc.scalar.activation(out=oT[64:P, hp, n0:n0 + 512],
                                     in_=pso_e[0:64, :], func=AF.Identity,
                                     scale=1.0 / 512.0)

        # ================= tail unit builders ================================
        mvb = {}

        def proj_resid(i, psT):
            """x2 = x + proj(oT) for token tile i, plus its LN2 stats."""
            for half, nw in ((0, 512), (1, 256)):
                ps = psT.tile([P, 512], FP32, tag="m1a" if half == 0 else "m1b",
                          name="ps_c")
                for k in range(0, KC, 2):
                    nc.tensor.matmul(ps[:, :nw],
                                     lhsT=oT[:, k:k + 2, i * P:(i + 1) * P],
                                     rhs=wproj_sb[:, k:k + 2,
                                                  half * 512:half * 512 + nw],
                                     start=(k == 0), stop=(k == KC - 2),
                                     perf_mode=DR)
                dst = x2[:, i, half * 512:half * 512 + nw]
                nc.vector.tensor_tensor(
                    out=dst, in0=ps[:, :nw],
                    in1=xfull[:, i, half * 512:half * 512 + nw], op=OP.add)
                if has_bproj:
                    nc.vector.tensor_tensor(
                        out=dst, in0=dst,
                        in1=bproj_bc[:, half * 512:half * 512 + nw], op=OP.add)
            g = i // 4
            if i % 4 == 0:
                mvb[g] = stats_p.tile([P, 4, 2], FP32, tag="mvb", name=f"mvb{g}")
            ln_stats(x2[:, i, :], mvb[g], i % 4)

        def ln2_group(g, psT=None):
            """LN2 normalize+transpose+cast for token tiles 4g..4g+3.
            psT given -> transpose on PE via the m1a/m1b psum slots."""
            rs = rsqrt_grp(mvb[g], 4)
            for j in range(4):
                i = 4 * g + j
                if psT is None:
                    ln_finish(i, j, x2[:, i, :], h2T, mvb[g], rs, "b", "act")
                    continue
                ht = stream.tile([P, C], BF, tag="ht", name=f"ht_b{i}")
                nc.vector.tensor_scalar(out=ht, in0=x2[:, i, :],
                                        scalar1=mvb[g][:, j, 0:1],
                                        scalar2=rs[:, j:j + 1],
                                        op0=OP.subtract, op1=OP.mult)
                for c in range(KC):
                    tpb = psT.tile([P, 1024], BF,
                                   tag="m1a" if c % 2 == 0 else "m1b",
                                   name="tr2_ps")
                    nc.tensor.transpose(tpb[:, 0:P], ht[:, c * P:(c + 1) * P],
                                        ident)
                    nc.scalar.activation(out=h2T[:, c, i * P:(i + 1) * P],
                                         in_=tpb[:, 0:P], func=AF.Identity)

        def mlp1_chunk(mc, nh, psT):
            nsl = slice(nh * 512, (nh + 1) * 512)
            ps = psT.tile([P, 512], FP32, tag="m1a" if mc % 2 == 0 else "m1b",
                          name="ps_m1")
            for k in range(0, KC, 2):
                nc.tensor.matmul(ps,
                                 lhsT=w1_sb[:, k:k + 2, mc * P:(mc + 1) * P],
                                 rhs=h2T[:, k:k + 2, nsl],
                                 start=(k == 0), stop=(k == KC - 2),
                                 perf_mode=DR)
            nc.scalar.activation(out=m1T[:, mc, nsl], in_=ps, func=AF.Silu,
                                 bias=b1s_sb[:, mc:mc + 1], scale=GELU_A)

        def mlp2_tile(i, psT):
            ot = stream.tile([P, C], FP32, tag="io_o", name=f"out_t{i}")
            for half, nw in ((0, 512), (1, 256)):
                ps = psT.tile([P, 512], FP32, tag="m1a" if half == 0 else "m1b",
                          name="ps_m2")
                for k in range(0, MHID, 2):
                    nc.tensor.matmul(ps[:, :nw],
                                     lhsT=m1T[:, k:k + 2, i * P:(i + 1) * P],
                                     rhs=w2_sb[:, k:k + 2,
                                               half * 512:half * 512 + nw],
                                     start=(k == 0), stop=(k == MHID - 2),
                                     perf_mode=DR)
                dst = ot[:, half * 512:half * 512 + nw]
                nc.vector.tensor_tensor(out=dst, in0=ps[:, :nw],
                                        in1=x2[:, i, half * 512:half * 512 + nw],
                                        op=OP.add)
                if has_b2:
                    nc.vector.tensor_tensor(
                        out=dst, in0=dst,
                        in1=b2_bc[:, half * 512:half * 512 + nw], op=OP.add)
            eng = nc.gpsimd if i % 2 == 0 else nc.sync
            eng.dma_start(out=out_d[i * P:(i + 1) * P, :], in_=ot)

        # ============ A2 (q/k chunk matmuls) + B1 / B2||T1 / T2 =============
        w1_sb = arena.tile([P, KC, HID], F8, tag="t18", name="w1_sb")
        w2_sb = arena.tile([P, MHID, C], F8, tag="t18b", name="w2_sb")

        with tc.tile_pool(name="psQK", bufs=2, space="PSUM") as psQK, \
             tc.tile_pool(name="psAV", bufs=1, space="PSUM") as psAV:
            with tc.tile_pool(name="psA2", bufs=1, space="PSUM") as psA2:
                for hp in range(HPAIRS):
                    # q then k chunk matmul; evac on ACT (identity + bias)
                    for which, dstT, col0 in ((0, qT, hp * P),
                                              (1, kTp, C + hp * P)):
                        ps2 = psA2.tile([P, N], FP32, tag="qk2", name="ps_qk2")
                        for half in range(2):
                            for k in range(0, KC, 2):
                                nc.tensor.matmul(
                                    ps2[:, half * 512:(half + 1) * 512],
                                    lhsT=wqkv_sb[:, k:k + 2, col0:col0 + P],
                                    rhs=hT[:, k:k + 2,
                                           half * 512:(half + 1) * 512],
                                    start=(k == 0), stop=(k == KC - 2),
                                    perf_mode=DR)
                        bcol = hp if which == 0 else KC + hp
                        if which == 0:
                            nc.scalar.activation(out=dstT[:, hp, :], in_=ps2,
                                                 func=AF.Identity,
                                                 bias=bqkv_sb[:, bcol:bcol + 1])
                        else:
                            nc.scalar.activation(
                                out=dstT[0:D, 2 * hp, :], in_=ps2[0:D, :],
                                func=AF.Identity,
                                bias=bqkv_sb[0:D, bcol:bcol + 1])
                            nc.scalar.activation(
                                out=dstT[D:P, 2 * hp + 1, :], in_=ps2[D:P, :],
                                func=AF.Identity,
                                bias=bqkv_sb[D:P, bcol:bcol + 1])
                    attn_block(hp, 0, psQK, psAV)
                    if hp == 0:
                        for k in range(KC):
                            nc.gpsimd.dma_start(out=w1_sb[:, k, :],
                                                in_=w1_d[k * P:(k + 1) * P, :])
                    if hp == 2:
                        for k in range(MHID):
                            nc.gpsimd.dma_start(out=w2_sb[:, k, :],
                                                in_=w2_d[k * P:(k + 1) * P, :])

            with tc.tile_pool(name="psT", bufs=1, space="PSUM") as psT:
                # B2 interleaved with T1 (tail on query half 0)
                tail_units = (
                    [lambda i=i: proj_resid(i, psT) for i in range(4)]
                    + [lambda: ln2_group(0)]
                    + [lambda mc=mc: mlp1_chunk(mc, 0, psT) for mc in range(MHID)]
                    + [lambda i=i: mlp2_tile(i, psT) for i in range(4)]
                )
                bounds = [0, 2, 5, 13, 21, 30, 31]
                for hp in range(HPAIRS):
                    attn_block(hp, 1, psQK, psAV)
                    for u in range(bounds[hp], bounds[hp + 1]):
                        tail_units[u]()
                # T2: tail on query half 1 (T1's last mlp2 tiles bridge the
                # proj/LN2 dependency chain)
                proj_resid(4, psT)
                proj_resid(5, psT)
                mlp2_tile(2, psT)
                proj_resid(6, psT)
                proj_resid(7, psT)
                mlp2_tile(3, psT)
                ln2_group(1, psT)
                for mc in range(MHID):
                    mlp1_chunk(mc, 1, psT)
                for i in range(4, NT):
                    mlp2_tile(i, psT)

    nc.finalize()
    return nc


def kernel(x, ln1_w, ln1_b, qkv_w, qkv_b, proj_w, proj_b, attn_bias,
           ls1, ln2_w, ln2_b, w1, b1, w2, b2, ls2):
    global LAST_EXEC_TIME_NS, LAST_TRACE_PATH, LAST_RESULTS
    from concourse.bass_utils import run_bass_kernel_spmd

    x = np.asarray(x, np.float32)
    f32 = lambda a: np.asarray(a, np.float32)
    ln1_w, ln1_b, qkv_w, qkv_b = f32(ln1_w), f32(ln1_b), f32(qkv_w), f32(qkv_b)
    proj_w, proj_b, ls1 = f32(proj_w), f32(proj_b), f32(ls1)
    ln2_w, ln2_b, w1, b1, w2, b2, ls2 = (f32(ln2_w), f32(ln2_b), f32(w1),
                                         f32(b1), f32(w2), f32(b2), f32(ls2))
    ab = float(np.asarray(attn_bias, np.float32))

    # ---- host-side weight folding (fp32, then cast to fp8) ----
    scale = D ** -0.5
    qkv_w_eff = qkv_w * ln1_w[None, :]
    bqkv_eff = qkv_b + qkv_w @ ln1_b
    wqkv_t = np.ascontiguousarray(qkv_w_eff.T)
    wqkv_t[:, :C] *= scale
    bqkv_eff = bqkv_eff.copy()
    bqkv_eff[:C] *= scale
    wproj_t = np.ascontiguousarray((proj_w * ls1[:, None]).T)
    bproj_eff = proj_b * ls1
    w1_t = np.ascontiguousarray((w1 * ln2_w[None, :]).T)
    b1_eff = (b1 + w1 @ ln2_b) * GELU_A          # silu input bias, pre-scaled
    w2_t = np.ascontiguousarray((w2 * ls2[:, None]).T) / GELU_A
    b2_eff = b2 * ls2

    has_vbias = bool(np.any(bqkv_eff[2 * C:] != 0.0))
    has_bproj = bool(np.any(bproj_eff != 0.0))
    has_b2 = bool(np.any(b2_eff != 0.0))

    nc = _build_program(ab, has_vbias, has_bproj, has_b2)

    import concourse.mybir as mybir
    F8NP = mybir.dt.np(mybir.dt.float8e4)
    shared = {
        "wqkv_t": wqkv_t.astype(F8NP),
        "bqkv": np.ascontiguousarray(
            bqkv_eff[:2 * C].reshape(2 * KC, P).T).astype(np.float32),
        "vbias": bqkv_eff[2 * C:].astype(np.float32),
        "wproj_t": wproj_t.astype(F8NP),
        "bproj": bproj_eff.astype(np.float32),
        "w1_t": w1_t.astype(F8NP),
        "b1s": np.ascontiguousarray(
            b1_eff.reshape(MHID, P).T).astype(np.float32),
        "w2_t": w2_t.astype(F8NP),
        "b2": b2_eff.astype(np.float32),
    }
    in_maps = [dict(shared, x=np.ascontiguousarray(x[c])) for c in range(NCORES)]

    trace = os.environ.get("KERNEL_TRACE", "0") == "1"
    res = run_bass_kernel_spmd(nc, in_maps, core_ids=list(range(NCORES)),
                               trace=trace)
    LAST_EXEC_TIME_NS = res.exec_time_ns
    LAST_RESULTS = res
    if res.instructions_and_trace is not None:
        LAST_TRACE_PATH = res.instructions_and_trace[1]
    return np.stack([r["out"] for r in res.results]).astype(np.float32)
